# revision 10
# baseline (speedup 1.0000x reference)
"""Multi-head attention on 8 Trainium2 cores — v2.

Sharding: core c handles batch b = c // 4 and a quad of 4 heads
(hq = c % 4) as two head-pairs p in {0,1}; pair p holds heads (2p, 2p+1)
on partition halves (rows 0-63 / 64-127) of khT[p]/qhT[p].

v2 main-loop changes vs baseline:
- Score matmuls of the two heads of a pair are interleaved so they run
  concurrently in disjoint PE row-groups (K=64 each, auto tile_position
  (0,0)/(64,0)) — halves score-stage PE time.
- softmax exp is split across engines: head-even on ACT (table exp),
  head-odd on DVE via Schraudolph bit-trick: uint16(x*184.665 + 16252)
  read as bf16 ~= exp(x) (1.8% rel std; constant bias cancels in the
  softmax normalization).
- scores psum is one (128, 2048) 4-bank tile per jp: quarters
  [e:jc0|e:jc1|o:jc0|o:jc1]; each exp call covers its head's 1024 cols.
- normalize: denom copy on ACT, reciprocal on DVE, broadcast + multiply
  on GpSimd; osb copies on ACT (frees DVE for exp).
"""

import numpy as np

B = 2
S = 2048
D = 1024
NH = 16
DH = 64
HEADS_PER_CORE = 4
N_CORES = 8

A16 = 128.0 / np.log(2.0)  # Schraudolph bf16 scale
B16 = 16248.6              # zero-mean offset for RNE float->uint16 convert

_NC = None


def _build():
    import concourse.bacc as bacc
    import concourse.tile as tile
    import concourse.mybir as mybir

    fp32 = mybir.dt.float32
    bf16 = mybir.dt.bfloat16
    u16 = mybir.dt.uint16
    add = mybir.AluOpType.add
    mult = mybir.AluOpType.mult
    Exp = mybir.ActivationFunctionType.Exp

    nc = bacc.Bacc("TRN2", target_bir_lowering=False)

    qT = nc.dram_tensor("qT", (D, S), bf16, kind="ExternalInput")
    kT = nc.dram_tensor("kT", (D, S), bf16, kind="ExternalInput")
    vT = nc.dram_tensor("vT", (D, S), bf16, kind="ExternalInput")
    wq = nc.dram_tensor("wq", (D, 256), bf16, kind="ExternalInput")
    wk = nc.dram_tensor("wk", (D, 256), bf16, kind="ExternalInput")
    wv = nc.dram_tensor("wv", (D, 256), bf16, kind="ExternalInput")
    wo = nc.dram_tensor("wo", (256, D), bf16, kind="ExternalInput")
    bq = nc.dram_tensor("bq", (256, 1), fp32, kind="ExternalInput")
    bk = nc.dram_tensor("bk", (256, 1), fp32, kind="ExternalInput")
    bv = nc.dram_tensor("bv", (1, 256), fp32, kind="ExternalInput")
    out = nc.dram_tensor("out", (S, D), fp32, kind="ExternalOutput")

    with tile.TileContext(nc) as tc:
        with tc.tile_pool(name="persist", bufs=1) as P:
            qhT = [P.tile((128, S), bf16, name=f"qhT{p}") for p in range(2)]
            khT = [P.tile((128, S), bf16, name=f"khT{p}") for p in range(2)]
            vh = [P.tile((128, 16 * 65), bf16, name=f"vh{h}") for h in range(4)]
            outnT = [P.tile((128, S), bf16, name=f"outnT{p}") for p in range(2)]
            wq_sb = P.tile((128, 8 * 256), bf16, name="wq_sb")
            wk_sb = P.tile((128, 8 * 256), bf16, name="wk_sb")
            wv_sb = P.tile((128, 8 * 256), bf16, name="wv_sb")
            wo_sb = [P.tile((128, D), bf16, name=f"wo_sb{p}") for p in range(2)]
            bq_sb = P.tile((128, 2), fp32, name="bq_sb")
            bk_sb = P.tile((128, 2), fp32, name="bk_sb")
            bv_row = P.tile((1, 256), fp32, name="bv_row")
            bv_bc = P.tile((128, 256), fp32, name="bv_bc")
            ones_f = P.tile((128, 16 * 65), fp32, name="ones_f")

            nc.gpsimd.memset(ones_f[:], 1.0)
            for h in range(4):
                nc.vector.tensor_scalar(
                    vh[h][:], ones_f[:], 1.0, None, op0=mult
                )

            # weights/biases on the ACT hwdge queue (ACT idle in stage A)
            for kc in range(8):
                nc.scalar.dma_start(
                    wk_sb[:, kc * 256:(kc + 1) * 256], wk[kc * 128:(kc + 1) * 128, :]
                )
            for p in range(2):
                nc.scalar.dma_start(bq_sb[:, p:p + 1], bq[p * 128:(p + 1) * 128, :])
                nc.scalar.dma_start(bk_sb[:, p:p + 1], bk[p * 128:(p + 1) * 128, :])
            nc.scalar.dma_start(bv_row[:], bv[:])
            nc.gpsimd.partition_broadcast(bv_bc[:], bv_row[:])

            with tc.tile_pool(name="xin", bufs=8) as XP:

                def load_x(xdram, ns, engs=None):
                    # split each 1MB chunk across issue queues so the
                    # per-dma_start issue cost (~600ns) doesn't gate the
                    # feed; gpsimd (idle in stage A) takes a quarter
                    engs = engs or (nc.sync, nc.sync, nc.sync, nc.gpsimd)
                    xt = XP.tile((128, 8 * 512), bf16, name="xt")
                    for kc in range(8):
                        engs[kc % len(engs)].dma_start(
                            xt[:, kc * 512:(kc + 1) * 512],
                            xdram[kc * 128:(kc + 1) * 128, ns * 512:(ns + 1) * 512],
                        )
                    return xt

                # ---- Stage A: khT (all), vh (all), qhT (qs=0 only) ----
                with tc.tile_pool(name="psA", bufs=2, space="PSUM") as PA, \
                     tc.tile_pool(name="psV", bufs=2, space="PSUM") as PV:

                    def proj_qk(xdram, w_sb, b_sb, dstT, ns, eng=None):
                        xt = load_x(xdram, ns, eng)
                        for p in range(2):
                            ps = PA.tile((128, 512), fp32, name="psa")
                            for kc in range(8):
                                nc.tensor.matmul(
                                    ps[:],
                                    w_sb[:, kc * 256 + p * 128:kc * 256 + (p + 1) * 128],
                                    xt[:, kc * 512:(kc + 1) * 512],
                                    start=(kc == 0),
                                    stop=(kc == 7),
                                )
                            nc.vector.tensor_scalar_add(
                                dstT[p][:, ns * 512:(ns + 1) * 512], ps[:], b_sb[:, p:p + 1]
                            )

                    for ns in range(4):
                        proj_qk(kT, wk_sb, bk_sb, khT, ns)
                    for kc in range(8):
                        nc.scalar.dma_start(
                            wv_sb[:, kc * 256:(kc + 1) * 256],
                            wv[kc * 128:(kc + 1) * 128, :],
                        )
                    for ns in range(4):
                        xt = load_x(vT, ns)
                        for jj in range(4):
                            jc = ns * 4 + jj
                            ps = PV.tile((128, 256), fp32, name="psv")
                            for kc in range(8):
                                nc.tensor.matmul(
                                    ps[:],
                                    xt[:, kc * 512 + jj * 128:kc * 512 + (jj + 1) * 128],
                                    wv_sb[:, kc * 256:(kc + 1) * 256],
                                    start=(kc == 0),
                                    stop=(kc == 7),
                                )
                            for h in range(4):
                                nc.vector.scalar_tensor_tensor(
                                    vh[h][:, jc * 65:jc * 65 + 64],
                                    ps[:, h * 64:(h + 1) * 64],
                                    1.0,
                                    bv_bc[:, h * 64:(h + 1) * 64],
                                    op0=mult,
                                    op1=add,
                                )
                    for kc in range(8):
                        nc.scalar.dma_start(
                            wq_sb[:, kc * 256:(kc + 1) * 256],
                            wq[kc * 128:(kc + 1) * 128, :],
                        )
                    proj_qk(qT, wq_sb, bq_sb, qhT, 0)
                    for p in range(2):
                        nc.scalar.dma_start(wo_sb[p][:], wo[p * 128:(p + 1) * 128, :])

                # ---- Main loop over (qs, p) blocks ----
                with tc.tile_pool(name="psS", bufs=1, space="PSUM") as PS, \
                     tc.tile_pool(name="psC", bufs=1, space="PSUM") as PC, \
                     tc.tile_pool(name="psDF", bufs=2, space="PSUM") as PD, \
                     tc.tile_pool(name="expA", bufs=3) as EA, \
                     tc.tile_pool(name="expD", bufs=3) as ED, \
                     tc.tile_pool(name="nrm", bufs=4) as NP, \
                     tc.tile_pool(name="outP", bufs=2) as OP:

                    def qh_filler(ns):
                        # DVE is exp-busy mid-loop: qh loads on sync only
                        xt = load_x(qT, ns, engs=(nc.sync,))
                        for p in range(2):
                            ps = PD.tile((128, 512), fp32, name="dps")
                            for kc in range(8):
                                nc.tensor.matmul(
                                    ps[:],
                                    wq_sb[:, kc * 256 + p * 128:kc * 256 + (p + 1) * 128],
                                    xt[:, kc * 512:(kc + 1) * 512],
                                    start=(kc == 0),
                                    stop=(kc == 7),
                                )
                                yield
                            nc.vector.tensor_scalar_add(
                                qhT[p][:, ns * 512:(ns + 1) * 512], ps[:], bq_sb[:, p:p + 1]
                            )

                    def d_filler(qs, tail=False):
                        for qq in range(4):
                            qc0 = qs * 512 + qq * 128
                            osb = OP.tile((128, D), fp32, name="osb")
                            for nsd in range(2):
                                dps = PD.tile((128, 512), fp32, name="dps")
                                for p in range(2):
                                    nc.tensor.matmul(
                                        dps[:],
                                        outnT[p][:, qc0:qc0 + 128],
                                        wo_sb[p][:, nsd * 512:(nsd + 1) * 512],
                                        start=(p == 0),
                                        stop=(p == 1),
                                    )
                                    yield
                                # psum->sbuf copy: ACT+DVE alternating at the
                                # tail (both idle), DVE mid-loop (ACT is
                                # exp-heavy)
                                if tail and nsd == 0:
                                    nc.scalar.copy(
                                        osb[:, nsd * 512:(nsd + 1) * 512], dps[:]
                                    )
                                else:
                                    nc.vector.tensor_scalar_add(
                                        osb[:, nsd * 512:(nsd + 1) * 512], dps[:], 0.0
                                    )
                            # row split: each dma_start lands on one DMA
                            # engine (~22.5 GB/s), so a single 512KB
                            # transfer would drain for ~23us
                            nway = 8 if tail else 4
                            step = 128 // nway
                            for rr in range(nway):
                                eng = nc.scalar if (tail and rr % 2) else nc.sync
                                eng.dma_start(
                                    out[qc0 + rr * step:qc0 + (rr + 1) * step, :],
                                    osb[rr * step:(rr + 1) * step, :],
                                )

                    def chain(*gens):
                        for g in gens:
                            yield from g

                    def emit_scores(p, qs, jp, spse, spso0, spso1):
                        # 4 MMs interleaved (e jc0), (o jc0), (e jc1), (o jc1):
                        # even head rows 0-63, odd rows 64-127 -> pairwise
                        # concurrent row-tiles. spso is split per-half so each
                        # bank frees right after its own exp (keeps the list
                        # scheduler from serializing the pairs).
                        spso = (spso0, spso1)
                        for half in range(2):
                            jc = jp * 2 + half
                            for po in range(2):  # 0 = even head, 1 = odd head
                                off = po * 64
                                dst = (spse[:, half * 512:(half + 1) * 512]
                                       if po == 0 else spso[half][:])
                                nc.tensor.matmul(
                                    dst,
                                    khT[p][off:off + 64, jc * 128:(jc + 1) * 128],
                                    qhT[p][off:off + 64, qs * 512:(qs + 1) * 512],
                                    start=True,
                                    stop=True,
                                )

                    def emit_exp(spse, spso0, spso1, exe, exo0, exo1):
                        # ACT: head-even (1024) + head-odd half1 (exact);
                        # DVE: head-odd half0 via Schraudolph bits
                        # uint16(x*A+B) viewed as bf16 (~1.8% rel std).
                        nc.scalar.activation(
                            exe[:], spse[:], Exp, bias=0.0, scale=0.125)
                        nc.vector.tensor_scalar(
                            exo0[:].bitcast(u16), spso0[:],
                            A16 * 0.125, B16, op0=mult, op1=add)
                        nc.scalar.activation(
                            exo1[:], spso1[:], Exp, bias=0.0, scale=0.125)

                    def emit_c(p, jp, cpse, cpso, exe, exo0, exo1):
                        exo = (exo0, exo1)
                        for half in range(2):
                            jc = jp * 2 + half
                            nc.tensor.matmul(
                                cpse[:],
                                vh[2 * p][:, jc * 65:(jc + 1) * 65],
                                exe[:, half * 512:(half + 1) * 512],
                                start=(jc == 0),
                                stop=(jc == 15),
                            )
                            nc.tensor.matmul(
                                cpso[:],
                                vh[2 * p + 1][:, jc * 65:(jc + 1) * 65],
                                exo[half][:],
                                start=(jc == 0),
                                stop=(jc == 15),
                            )

                    norm_q = []

                    def normalize(p, po, qs, cps):
                        off = po * 64
                        den = NP.tile((1, 512), fp32, name="den")
                        # recip needs SBUF input; denom row copy on ACT
                        nc.scalar.copy(den[:], cps[64:65, :])
                        rec = NP.tile((1, 512), fp32, name="rec")
                        nc.vector.reciprocal_approx_fast(rec[:], den[:])
                        rbc = NP.tile((64, 512), fp32, name="rbc")
                        nc.gpsimd.partition_broadcast(rbc[:], rec[:])

                        # the final multiplies are deferred one consume so
                        # they don't bunch up in the DVE queue ahead of the
                        # next slot's exp (they still precede the next
                        # block's C writes into the same cps banks)
                        def mult_fin():
                            nc.vector.scalar_tensor_tensor(
                                outnT[p][off:off + 64, qs * 512:(qs + 1) * 512],
                                cps[0:64, :],
                                1.0,
                                rbc[:],
                                op0=mult,
                                op1=mult,
                            )
                        norm_q.append(mult_fin)

                    # block schedule: (qs, p) with fillers
                    blocks = [(qs, p) for qs in range(4) for p in range(2)]
                    fillers = {
                        0: chain(qh_filler(1)),
                        1: chain(qh_filler(2)),
                        2: chain(qh_filler(3)),
                        3: chain(d_filler(0)),
                        5: chain(d_filler(1)),
                        7: chain(d_filler(2)),
                    }

                    pending = [None]

                    def consume_pending():
                        # deferred normalize multiplies first: they must
                        # precede any new C matmul into the recycled banks
                        while norm_q:
                            norm_q.pop(0)()
                        if pending[0] is not None:
                            (pp, pqs, pjp, pcpse, pcpso, pexs) = pending[0]
                            emit_c(pp, pjp, pcpse, pcpso, *pexs)
                            if pjp == 7:
                                normalize(pp, 0, pqs, pcpse)
                                normalize(pp, 1, pqs, pcpso)
                            pending[0] = None

                    for bi, (qs, p) in enumerate(blocks):
                        filler = fillers.get(bi)
                        # consume before re-allocating from the 2-deep cps
                        # pool: the pending block's C/normalize must be
                        # emitted before its banks are handed out again
                        consume_pending()
                        cpse = PC.tile((65, 512), fp32, name="cpse")
                        cpso = PC.tile((65, 512), fp32, name="cpso")
                        for jp in range(8):
                            spse = PS.tile((128, 1024), fp32, name="spse")
                            spso0 = PS.tile((128, 512), fp32, name="spso0")
                            spso1 = PS.tile((128, 512), fp32, name="spso1")
                            emit_scores(p, qs, jp, spse, spso0, spso1)
                            exe = EA.tile((128, 1024), bf16, name="exe")
                            exo0 = ED.tile((128, 512), bf16, name="exo0")
                            exo1 = ED.tile((128, 512), bf16, name="exo1")
                            emit_exp(spse, spso0, spso1, exe, exo0, exo1)
                            consume_pending()
                            if filler is not None:
                                next(filler, None)
                                next(filler, None)
                            pending[0] = (p, qs, jp, cpse, cpso,
                                          (exe, exo0, exo1))
                        if filler is not None:
                            for _ in filler:
                                pass
                    consume_pending()
                    while norm_q:
                        norm_q.pop(0)()
                    for _ in d_filler(3, tail=True):
                        pass

    nc.compile()
    return nc


def _get_nc():
    global _NC
    if _NC is None:
        _NC = _build()
    return _NC


def run(inputs, trace=False, trace_cores=None):
    from concourse.bass_utils import run_bass_kernel_spmd

    q = np.asarray(inputs["q"], np.float32)
    k = np.asarray(inputs["k"], np.float32)
    v = np.asarray(inputs["v"], np.float32)
    w_q = np.asarray(inputs["w_q"], np.float32)
    w_k = np.asarray(inputs["w_k"], np.float32)
    w_v = np.asarray(inputs["w_v"], np.float32)
    w_out = np.asarray(inputs["w_out"], np.float32)
    b_q = np.asarray(inputs["b_q"], np.float32)
    b_k = np.asarray(inputs["b_k"], np.float32)
    b_v = np.asarray(inputs["b_v"], np.float32)
    b_out = np.asarray(inputs["b_out"], np.float32)

    import ml_dtypes
    bf16 = ml_dtypes.bfloat16

    xT = {b: {} for b in range(B)}
    for b in range(B):
        xT[b]["qT"] = np.ascontiguousarray(q[b].T.astype(bf16))
        xT[b]["kT"] = np.ascontiguousarray(k[b].T.astype(bf16))
        xT[b]["vT"] = np.ascontiguousarray(v[b].T.astype(bf16))

    in_maps = []
    for c in range(N_CORES):
        b, hq = c // 4, c % 4
        rows = slice(hq * 256, (hq + 1) * 256)
        in_maps.append({
            "qT": xT[b]["qT"],
            "kT": xT[b]["kT"],
            "vT": xT[b]["vT"],
            "wq": np.ascontiguousarray(w_q[rows, :].T.astype(bf16)),
            "wk": np.ascontiguousarray(w_k[rows, :].T.astype(bf16)),
            "wv": np.ascontiguousarray(w_v[rows, :].T.astype(bf16)),
            "wo": np.ascontiguousarray(w_out[:, rows].T.astype(bf16)),
            "bq": np.ascontiguousarray(b_q[rows].reshape(256, 1)),
            "bk": np.ascontiguousarray(b_k[rows].reshape(256, 1)),
            "bv": np.ascontiguousarray(b_v[rows].reshape(1, 256)),
        })

    nc = _get_nc()
    res = run_bass_kernel_spmd(
        nc, in_maps, core_ids=list(range(N_CORES)), trace=trace,
        trace_cores=trace_cores,
    )
    full = np.zeros((B, S, D), np.float32)
    for c in range(N_CORES):
        full[c // 4] += np.asarray(res.results[c]["out"])
    full += b_out.reshape(1, 1, D)
    return full, res.exec_time_ns


def kernel(**inputs):
    return run(inputs, trace=False)[0]


# revision 12
# speedup vs baseline: 1.0598x; 1.0598x over previous
"""Multi-head attention on 8 Trainium2 cores — v2.

Sharding: core c handles batch b = c // 4 and a quad of 4 heads
(hq = c % 4) as two head-pairs p in {0,1}; pair p holds heads (2p, 2p+1)
on partition halves (rows 0-63 / 64-127) of khT[p]/qhT[p].

v2 main-loop changes vs baseline:
- Score matmuls of the two heads of a pair are interleaved so they run
  concurrently in disjoint PE row-groups (K=64 each, auto tile_position
  (0,0)/(64,0)) — halves score-stage PE time.
- softmax exp is split across engines: head-even on ACT (table exp),
  head-odd on DVE via Schraudolph bit-trick: uint16(x*184.665 + 16252)
  read as bf16 ~= exp(x) (1.8% rel std; constant bias cancels in the
  softmax normalization).
- scores psum is one (128, 2048) 4-bank tile per jp: quarters
  [e:jc0|e:jc1|o:jc0|o:jc1]; each exp call covers its head's 1024 cols.
- normalize: denom copy on ACT, reciprocal on DVE, broadcast + multiply
  on GpSimd; osb copies on ACT (frees DVE for exp).
"""

import numpy as np

B = 2
S = 2048
D = 1024
NH = 16
DH = 64
HEADS_PER_CORE = 4
N_CORES = 8

A16 = 128.0 / np.log(2.0)  # Schraudolph bf16 scale
B16 = 16248.6              # zero-mean offset for RNE float->uint16 convert

_NC = None


def _build():
    import concourse.bacc as bacc
    import concourse.tile as tile
    import concourse.mybir as mybir

    fp32 = mybir.dt.float32
    bf16 = mybir.dt.bfloat16
    u16 = mybir.dt.uint16
    add = mybir.AluOpType.add
    mult = mybir.AluOpType.mult
    Exp = mybir.ActivationFunctionType.Exp

    nc = bacc.Bacc("TRN2", target_bir_lowering=False)

    qT = nc.dram_tensor("qT", (D, S), bf16, kind="ExternalInput")
    kT = nc.dram_tensor("kT", (D, S), bf16, kind="ExternalInput")
    vT = nc.dram_tensor("vT", (D, S), bf16, kind="ExternalInput")
    wq = nc.dram_tensor("wq", (D, 256), bf16, kind="ExternalInput")
    wk = nc.dram_tensor("wk", (D, 256), bf16, kind="ExternalInput")
    wv = nc.dram_tensor("wv", (D, 256), bf16, kind="ExternalInput")
    wo = nc.dram_tensor("wo", (256, D), bf16, kind="ExternalInput")
    bq = nc.dram_tensor("bq", (256, 1), fp32, kind="ExternalInput")
    bk = nc.dram_tensor("bk", (256, 1), fp32, kind="ExternalInput")
    bv = nc.dram_tensor("bv", (1, 256), fp32, kind="ExternalInput")
    out = nc.dram_tensor("out", (S, D), fp32, kind="ExternalOutput")

    with tile.TileContext(nc) as tc:
        with tc.tile_pool(name="persist", bufs=1) as P:
            qhT = [P.tile((128, S), bf16, name=f"qhT{p}") for p in range(2)]
            khT = [P.tile((128, S), bf16, name=f"khT{p}") for p in range(2)]
            vh = [P.tile((128, 16 * 65), bf16, name=f"vh{h}") for h in range(4)]
            outnT = [P.tile((128, S), bf16, name=f"outnT{p}") for p in range(2)]
            wq_sb = P.tile((128, 8 * 256), bf16, name="wq_sb")
            wk_sb = P.tile((128, 8 * 256), bf16, name="wk_sb")
            wv_sb = P.tile((128, 8 * 256), bf16, name="wv_sb")
            wo_sb = [P.tile((128, D), bf16, name=f"wo_sb{p}") for p in range(2)]
            bq_sb = P.tile((128, 2), fp32, name="bq_sb")
            bk_sb = P.tile((128, 2), fp32, name="bk_sb")
            bv_row = P.tile((1, 256), fp32, name="bv_row")
            bv_bc = P.tile((128, 256), fp32, name="bv_bc")
            ones_f = P.tile((128, 16 * 65), fp32, name="ones_f")

            nc.gpsimd.memset(ones_f[:], 1.0)
            for h in range(4):
                nc.vector.tensor_scalar(
                    vh[h][:], ones_f[:], 1.0, None, op0=mult
                )

            # weights/biases on the ACT hwdge queue (ACT idle in stage A)
            for kc in range(8):
                nc.scalar.dma_start(
                    wk_sb[:, kc * 256:(kc + 1) * 256], wk[kc * 128:(kc + 1) * 128, :]
                )
            for p in range(2):
                nc.scalar.dma_start(bq_sb[:, p:p + 1], bq[p * 128:(p + 1) * 128, :])
                nc.scalar.dma_start(bk_sb[:, p:p + 1], bk[p * 128:(p + 1) * 128, :])
            nc.scalar.dma_start(bv_row[:], bv[:])
            nc.gpsimd.partition_broadcast(bv_bc[:], bv_row[:])

            with tc.tile_pool(name="xin", bufs=8) as XP:

                def load_x(xdram, ns, engs=None):
                    # split each 1MB chunk across issue queues so the
                    # per-dma_start issue cost (~600ns) doesn't gate the
                    # feed; gpsimd (idle in stage A) takes a quarter
                    engs = engs or (nc.sync, nc.sync, nc.sync, nc.gpsimd)
                    xt = XP.tile((128, 8 * 512), bf16, name="xt")
                    for kc in range(8):
                        engs[kc % len(engs)].dma_start(
                            xt[:, kc * 512:(kc + 1) * 512],
                            xdram[kc * 128:(kc + 1) * 128, ns * 512:(ns + 1) * 512],
                        )
                    return xt

                # ---- Stage A: khT (all), vh (all), qhT (qs=0 only) ----
                with tc.tile_pool(name="psA", bufs=2, space="PSUM") as PA, \
                     tc.tile_pool(name="psV", bufs=2, space="PSUM") as PV:

                    def proj_qk(xdram, w_sb, b_sb, dstT, ns, eng=None):
                        xt = load_x(xdram, ns, eng)
                        for p in range(2):
                            ps = PA.tile((128, 512), fp32, name="psa")
                            for kc in range(8):
                                nc.tensor.matmul(
                                    ps[:],
                                    w_sb[:, kc * 256 + p * 128:kc * 256 + (p + 1) * 128],
                                    xt[:, kc * 512:(kc + 1) * 512],
                                    start=(kc == 0),
                                    stop=(kc == 7),
                                )
                            nc.vector.tensor_scalar_add(
                                dstT[p][:, ns * 512:(ns + 1) * 512], ps[:], b_sb[:, p:p + 1]
                            )

                    def vproj_chunk(ns, PVpool):
                        xt = load_x(vT, ns)
                        for jj in range(4):
                            jc = ns * 4 + jj
                            # shares the main-loop pool's "dps" slot shape
                            ps = PVpool.tile((128, 512), fp32, name="dps")
                            ps = ps[:, 0:256]
                            for kc in range(8):
                                nc.tensor.matmul(
                                    ps[:],
                                    xt[:, kc * 512 + jj * 128:kc * 512 + (jj + 1) * 128],
                                    wv_sb[:, kc * 256:(kc + 1) * 256],
                                    start=(kc == 0),
                                    stop=(kc == 7),
                                )
                                if kc % 4 == 3:
                                    yield
                            for h in range(4):
                                nc.vector.scalar_tensor_tensor(
                                    vh[h][:, jc * 65:jc * 65 + 64],
                                    ps[:, h * 64:(h + 1) * 64],
                                    1.0,
                                    bv_bc[:, h * 64:(h + 1) * 64],
                                    op0=mult,
                                    op1=add,
                                )

                    # k -> q0 -> v(ns0, ns1); v(ns2, ns3) overlap into the
                    # first main block as filler
                    for ns in range(4):
                        proj_qk(kT, wk_sb, bk_sb, khT, ns)
                    for kc in range(8):
                        nc.scalar.dma_start(
                            wq_sb[:, kc * 256:(kc + 1) * 256],
                            wq[kc * 128:(kc + 1) * 128, :],
                        )
                    for kc in range(8):
                        nc.scalar.dma_start(
                            wv_sb[:, kc * 256:(kc + 1) * 256],
                            wv[kc * 128:(kc + 1) * 128, :],
                        )
                    proj_qk(qT, wq_sb, bq_sb, qhT, 0)
                    for ns in range(2):
                        for _ in vproj_chunk(ns, PV):
                            pass
                    for p in range(2):
                        nc.scalar.dma_start(wo_sb[p][:], wo[p * 128:(p + 1) * 128, :])

                # ---- Main loop over (qs, p) blocks ----
                with tc.tile_pool(name="psS", bufs=1, space="PSUM") as PS, \
                     tc.tile_pool(name="psC", bufs=1, space="PSUM") as PC, \
                     tc.tile_pool(name="psDF", bufs=2, space="PSUM") as PD, \
                     tc.tile_pool(name="expA", bufs=3) as EA, \
                     tc.tile_pool(name="expD", bufs=3) as ED, \
                     tc.tile_pool(name="nrm", bufs=4) as NP, \
                     tc.tile_pool(name="outP", bufs=2) as OP:

                    def qh_filler(ns):
                        # DVE is exp-busy mid-loop: qh loads on sync only
                        xt = load_x(qT, ns, engs=(nc.sync,))
                        for p in range(2):
                            ps = PD.tile((128, 512), fp32, name="dps")
                            for kc in range(8):
                                nc.tensor.matmul(
                                    ps[:],
                                    wq_sb[:, kc * 256 + p * 128:kc * 256 + (p + 1) * 128],
                                    xt[:, kc * 512:(kc + 1) * 512],
                                    start=(kc == 0),
                                    stop=(kc == 7),
                                )
                                yield
                            nc.vector.tensor_scalar_add(
                                qhT[p][:, ns * 512:(ns + 1) * 512], ps[:], bq_sb[:, p:p + 1]
                            )

                    def d_filler(qs, tail=False):
                        for qq in range(4):
                            qc0 = qs * 512 + qq * 128
                            osb = OP.tile((128, D), fp32, name="osb")
                            for nsd in range(2):
                                dps = PD.tile((128, 512), fp32, name="dps")
                                for p in range(2):
                                    nc.tensor.matmul(
                                        dps[:],
                                        outnT[p][:, qc0:qc0 + 128],
                                        wo_sb[p][:, nsd * 512:(nsd + 1) * 512],
                                        start=(p == 0),
                                        stop=(p == 1),
                                    )
                                    yield
                                # psum->sbuf copy: ACT+DVE alternating at the
                                # tail (both idle), DVE mid-loop (ACT is
                                # exp-heavy)
                                if tail and nsd == 0:
                                    nc.scalar.copy(
                                        osb[:, nsd * 512:(nsd + 1) * 512], dps[:]
                                    )
                                else:
                                    nc.vector.tensor_scalar_add(
                                        osb[:, nsd * 512:(nsd + 1) * 512], dps[:], 0.0
                                    )
                            # row split: each dma_start lands on one DMA
                            # engine (~22.5 GB/s), so a single 512KB
                            # transfer would drain for ~23us
                            for rr in range(4):
                                eng = nc.scalar if (tail and rr % 2) else nc.sync
                                eng.dma_start(
                                    out[qc0 + rr * 32:qc0 + (rr + 1) * 32, :],
                                    osb[rr * 32:(rr + 1) * 32, :],
                                )

                    def chain(*gens):
                        for g in gens:
                            yield from g

                    def emit_scores(p, qs, jp, spse, spso0, spso1):
                        # 4 MMs interleaved (e jc0), (o jc0), (e jc1), (o jc1):
                        # even head rows 0-63, odd rows 64-127 -> pairwise
                        # concurrent row-tiles. spso is split per-half so each
                        # bank frees right after its own exp (keeps the list
                        # scheduler from serializing the pairs).
                        spso = (spso0, spso1)
                        for half in range(2):
                            jc = jp * 2 + half
                            for po in range(2):  # 0 = even head, 1 = odd head
                                off = po * 64
                                dst = (spse[:, half * 512:(half + 1) * 512]
                                       if po == 0 else spso[half][:])
                                nc.tensor.matmul(
                                    dst,
                                    khT[p][off:off + 64, jc * 128:(jc + 1) * 128],
                                    qhT[p][off:off + 64, qs * 512:(qs + 1) * 512],
                                    start=True,
                                    stop=True,
                                )

                    def emit_exp(spse, spso0, spso1, exe, exo0, exo1):
                        # ACT: head-even (1024) + head-odd half1 (exact);
                        # DVE: head-odd half0 via Schraudolph bits
                        # uint16(x*A+B) viewed as bf16 (~1.8% rel std).
                        nc.scalar.activation(
                            exe[:], spse[:], Exp, bias=0.0, scale=0.125)
                        nc.vector.tensor_scalar(
                            exo0[:].bitcast(u16), spso0[:],
                            A16 * 0.125, B16, op0=mult, op1=add)
                        nc.scalar.activation(
                            exo1[:], spso1[:], Exp, bias=0.0, scale=0.125)

                    def emit_c(p, jp, cpse, cpso, exe, exo0, exo1):
                        exo = (exo0, exo1)
                        for half in range(2):
                            jc = jp * 2 + half
                            nc.tensor.matmul(
                                cpse[:],
                                vh[2 * p][:, jc * 65:(jc + 1) * 65],
                                exe[:, half * 512:(half + 1) * 512],
                                start=(jc == 0),
                                stop=(jc == 15),
                            )
                            nc.tensor.matmul(
                                cpso[:],
                                vh[2 * p + 1][:, jc * 65:(jc + 1) * 65],
                                exo[half][:],
                                start=(jc == 0),
                                stop=(jc == 15),
                            )

                    norm_q = []

                    def normalize(p, po, qs, cps):
                        off = po * 64
                        den = NP.tile((1, 512), fp32, name="den")
                        # recip needs SBUF input; denom row copy on ACT
                        nc.scalar.copy(den[:], cps[64:65, :])
                        rec = NP.tile((1, 512), fp32, name="rec")
                        nc.vector.reciprocal_approx_fast(rec[:], den[:])
                        rbc = NP.tile((64, 512), fp32, name="rbc")
                        nc.gpsimd.partition_broadcast(rbc[:], rec[:])

                        # the final multiplies are deferred one consume so
                        # they don't bunch up in the DVE queue ahead of the
                        # next slot's exp (they still precede the next
                        # block's C writes into the same cps banks)
                        def mult_fin():
                            nc.vector.scalar_tensor_tensor(
                                outnT[p][off:off + 64, qs * 512:(qs + 1) * 512],
                                cps[0:64, :],
                                1.0,
                                rbc[:],
                                op0=mult,
                                op1=mult,
                            )
                        norm_q.append(mult_fin)

                    # block schedule: (qs, p) with fillers
                    blocks = [(qs, p) for qs in range(4) for p in range(2)]
                    fillers = {
                        0: chain(vproj_chunk(2, PD), vproj_chunk(3, PD)),
                        1: chain(qh_filler(1)),
                        2: chain(qh_filler(2)),
                        3: chain(d_filler(0)),
                        4: chain(qh_filler(3)),
                        5: chain(d_filler(1)),
                        7: chain(d_filler(2)),
                    }

                    pending = [None]

                    def consume_pending():
                        # deferred normalize multiplies first: they must
                        # precede any new C matmul into the recycled banks
                        while norm_q:
                            norm_q.pop(0)()
                        if pending[0] is not None:
                            (pp, pqs, pjp, pcpse, pcpso, pexs) = pending[0]
                            emit_c(pp, pjp, pcpse, pcpso, *pexs)
                            if pjp == 7:
                                normalize(pp, 0, pqs, pcpse)
                                normalize(pp, 1, pqs, pcpso)
                            pending[0] = None

                    for bi, (qs, p) in enumerate(blocks):
                        filler = fillers.get(bi)
                        # consume before re-allocating from the 2-deep cps
                        # pool: the pending block's C/normalize must be
                        # emitted before its banks are handed out again
                        consume_pending()
                        cpse = PC.tile((65, 512), fp32, name="cpse")
                        cpso = PC.tile((65, 512), fp32, name="cpso")
                        for jp in range(8):
                            spse = PS.tile((128, 1024), fp32, name="spse")
                            spso0 = PS.tile((128, 512), fp32, name="spso0")
                            spso1 = PS.tile((128, 512), fp32, name="spso1")
                            emit_scores(p, qs, jp, spse, spso0, spso1)
                            exe = EA.tile((128, 1024), bf16, name="exe")
                            exo0 = ED.tile((128, 512), bf16, name="exo0")
                            exo1 = ED.tile((128, 512), bf16, name="exo1")
                            emit_exp(spse, spso0, spso1, exe, exo0, exo1)
                            consume_pending()
                            if filler is not None:
                                next(filler, None)
                                next(filler, None)
                            pending[0] = (p, qs, jp, cpse, cpso,
                                          (exe, exo0, exo1))
                        if filler is not None:
                            for _ in filler:
                                pass
                    consume_pending()
                    while norm_q:
                        norm_q.pop(0)()
                    for _ in d_filler(3, tail=True):
                        pass

    nc.compile()
    return nc


def _get_nc():
    global _NC
    if _NC is None:
        _NC = _build()
    return _NC


def run(inputs, trace=False, trace_cores=None):
    from concourse.bass_utils import run_bass_kernel_spmd

    q = np.asarray(inputs["q"], np.float32)
    k = np.asarray(inputs["k"], np.float32)
    v = np.asarray(inputs["v"], np.float32)
    w_q = np.asarray(inputs["w_q"], np.float32)
    w_k = np.asarray(inputs["w_k"], np.float32)
    w_v = np.asarray(inputs["w_v"], np.float32)
    w_out = np.asarray(inputs["w_out"], np.float32)
    b_q = np.asarray(inputs["b_q"], np.float32)
    b_k = np.asarray(inputs["b_k"], np.float32)
    b_v = np.asarray(inputs["b_v"], np.float32)
    b_out = np.asarray(inputs["b_out"], np.float32)

    import ml_dtypes
    bf16 = ml_dtypes.bfloat16

    xT = {b: {} for b in range(B)}
    for b in range(B):
        xT[b]["qT"] = np.ascontiguousarray(q[b].T.astype(bf16))
        xT[b]["kT"] = np.ascontiguousarray(k[b].T.astype(bf16))
        xT[b]["vT"] = np.ascontiguousarray(v[b].T.astype(bf16))

    in_maps = []
    for c in range(N_CORES):
        b, hq = c // 4, c % 4
        rows = slice(hq * 256, (hq + 1) * 256)
        in_maps.append({
            "qT": xT[b]["qT"],
            "kT": xT[b]["kT"],
            "vT": xT[b]["vT"],
            "wq": np.ascontiguousarray(w_q[rows, :].T.astype(bf16)),
            "wk": np.ascontiguousarray(w_k[rows, :].T.astype(bf16)),
            "wv": np.ascontiguousarray(w_v[rows, :].T.astype(bf16)),
            "wo": np.ascontiguousarray(w_out[:, rows].T.astype(bf16)),
            "bq": np.ascontiguousarray(b_q[rows].reshape(256, 1)),
            "bk": np.ascontiguousarray(b_k[rows].reshape(256, 1)),
            "bv": np.ascontiguousarray(b_v[rows].reshape(1, 256)),
        })

    nc = _get_nc()
    res = run_bass_kernel_spmd(
        nc, in_maps, core_ids=list(range(N_CORES)), trace=trace,
        trace_cores=trace_cores,
    )
    full = np.zeros((B, S, D), np.float32)
    for c in range(N_CORES):
        full[c // 4] += np.asarray(res.results[c]["out"])
    full += b_out.reshape(1, 1, D)
    return full, res.exec_time_ns


def kernel(**inputs):
    return run(inputs, trace=False)[0]


# revision 14
# speedup vs baseline: 1.0603x; 1.0005x over previous
"""Multi-head attention on 8 Trainium2 cores — v2.

Sharding: core c handles batch b = c // 4 and a quad of 4 heads
(hq = c % 4) as two head-pairs p in {0,1}; pair p holds heads (2p, 2p+1)
on partition halves (rows 0-63 / 64-127) of khT[p]/qhT[p].

v2 main-loop changes vs baseline:
- Score matmuls of the two heads of a pair are interleaved so they run
  concurrently in disjoint PE row-groups (K=64 each, auto tile_position
  (0,0)/(64,0)) — halves score-stage PE time.
- softmax exp is split across engines: head-even on ACT (table exp),
  head-odd on DVE via Schraudolph bit-trick: uint16(x*184.665 + 16252)
  read as bf16 ~= exp(x) (1.8% rel std; constant bias cancels in the
  softmax normalization).
- scores psum is one (128, 2048) 4-bank tile per jp: quarters
  [e:jc0|e:jc1|o:jc0|o:jc1]; each exp call covers its head's 1024 cols.
- normalize: denom copy on ACT, reciprocal on DVE, broadcast + multiply
  on GpSimd; osb copies on ACT (frees DVE for exp).
"""

import numpy as np

B = 2
S = 2048
D = 1024
NH = 16
DH = 64
HEADS_PER_CORE = 4
N_CORES = 8

A16 = 128.0 / np.log(2.0)  # Schraudolph bf16 scale
B16 = 16248.6              # zero-mean offset for RNE float->uint16 convert

_NC = None


def _build():
    import concourse.bacc as bacc
    import concourse.tile as tile
    import concourse.mybir as mybir

    fp32 = mybir.dt.float32
    bf16 = mybir.dt.bfloat16
    u16 = mybir.dt.uint16
    add = mybir.AluOpType.add
    mult = mybir.AluOpType.mult
    Exp = mybir.ActivationFunctionType.Exp

    nc = bacc.Bacc("TRN2", target_bir_lowering=False)

    qT = nc.dram_tensor("qT", (D, S), bf16, kind="ExternalInput")
    kT = nc.dram_tensor("kT", (D, S), bf16, kind="ExternalInput")
    vT = nc.dram_tensor("vT", (D, S), bf16, kind="ExternalInput")
    wq = nc.dram_tensor("wq", (D, 256), bf16, kind="ExternalInput")
    wk = nc.dram_tensor("wk", (D, 256), bf16, kind="ExternalInput")
    wv = nc.dram_tensor("wv", (D, 256), bf16, kind="ExternalInput")
    wo = nc.dram_tensor("wo", (256, D), bf16, kind="ExternalInput")
    bq = nc.dram_tensor("bq", (256, 1), fp32, kind="ExternalInput")
    bk = nc.dram_tensor("bk", (256, 1), fp32, kind="ExternalInput")
    bv = nc.dram_tensor("bv", (1, 256), fp32, kind="ExternalInput")
    out = nc.dram_tensor("out", (S, D), fp32, kind="ExternalOutput")

    with tile.TileContext(nc) as tc:
        with tc.tile_pool(name="persist", bufs=1) as P:
            qhT = [P.tile((128, S), bf16, name=f"qhT{p}") for p in range(2)]
            khT = [P.tile((128, S), bf16, name=f"khT{p}") for p in range(2)]
            vh = [P.tile((128, 16 * 65), bf16, name=f"vh{h}") for h in range(4)]
            outnT = [P.tile((128, S), bf16, name=f"outnT{p}") for p in range(2)]
            wq_sb = P.tile((128, 8 * 256), bf16, name="wq_sb")
            wk_sb = P.tile((128, 8 * 256), bf16, name="wk_sb")
            wv_sb = P.tile((128, 8 * 256), bf16, name="wv_sb")
            wo_sb = [P.tile((128, D), bf16, name=f"wo_sb{p}") for p in range(2)]
            bq_sb = P.tile((128, 2), fp32, name="bq_sb")
            bk_sb = P.tile((128, 2), fp32, name="bk_sb")
            bv_row = P.tile((1, 256), fp32, name="bv_row")
            bv_bc = P.tile((128, 256), fp32, name="bv_bc")
            ones_f = P.tile((128, 16 * 65), fp32, name="ones_f")

            nc.gpsimd.memset(ones_f[:], 1.0)
            for h in range(4):
                nc.vector.tensor_scalar(
                    vh[h][:], ones_f[:], 1.0, None, op0=mult
                )

            # weights/biases on the ACT hwdge queue (ACT idle in stage A)
            for kc in range(8):
                nc.scalar.dma_start(
                    wk_sb[:, kc * 256:(kc + 1) * 256], wk[kc * 128:(kc + 1) * 128, :]
                )
            for p in range(2):
                nc.scalar.dma_start(bq_sb[:, p:p + 1], bq[p * 128:(p + 1) * 128, :])
                nc.scalar.dma_start(bk_sb[:, p:p + 1], bk[p * 128:(p + 1) * 128, :])
            nc.scalar.dma_start(bv_row[:], bv[:])
            nc.gpsimd.partition_broadcast(bv_bc[:], bv_row[:])

            with tc.tile_pool(name="xin", bufs=8) as XP:

                def load_x(xdram, ns, engs=None):
                    # split each 1MB chunk across issue queues so the
                    # per-dma_start issue cost (~600ns) doesn't gate the
                    # feed; gpsimd (idle in stage A) takes a quarter
                    engs = engs or (nc.sync, nc.sync, nc.sync, nc.gpsimd)
                    xt = XP.tile((128, 8 * 512), bf16, name="xt")
                    for kc in range(8):
                        engs[kc % len(engs)].dma_start(
                            xt[:, kc * 512:(kc + 1) * 512],
                            xdram[kc * 128:(kc + 1) * 128, ns * 512:(ns + 1) * 512],
                        )
                    return xt

                # ---- Stage A: khT (all), vh (all), qhT (qs=0 only) ----
                with tc.tile_pool(name="psA", bufs=2, space="PSUM") as PA, \
                     tc.tile_pool(name="psV", bufs=2, space="PSUM") as PV:

                    def proj_qk(xdram, w_sb, b_sb, dstT, ns, eng=None):
                        xt = load_x(xdram, ns, eng)
                        for p in range(2):
                            ps = PA.tile((128, 512), fp32, name="psa")
                            for kc in range(8):
                                nc.tensor.matmul(
                                    ps[:],
                                    w_sb[:, kc * 256 + p * 128:kc * 256 + (p + 1) * 128],
                                    xt[:, kc * 512:(kc + 1) * 512],
                                    start=(kc == 0),
                                    stop=(kc == 7),
                                )
                            nc.vector.tensor_scalar_add(
                                dstT[p][:, ns * 512:(ns + 1) * 512], ps[:], b_sb[:, p:p + 1]
                            )

                    def vproj_chunk(ns, PVpool):
                        xt = load_x(vT, ns)
                        for jj in range(4):
                            jc = ns * 4 + jj
                            # shares the main-loop pool's "dps" slot shape
                            ps = PVpool.tile((128, 512), fp32, name="dps")
                            ps = ps[:, 0:256]
                            for kc in range(8):
                                nc.tensor.matmul(
                                    ps[:],
                                    xt[:, kc * 512 + jj * 128:kc * 512 + (jj + 1) * 128],
                                    wv_sb[:, kc * 256:(kc + 1) * 256],
                                    start=(kc == 0),
                                    stop=(kc == 7),
                                )
                                if kc % 4 == 3:
                                    yield
                            for h in range(4):
                                nc.vector.scalar_tensor_tensor(
                                    vh[h][:, jc * 65:jc * 65 + 64],
                                    ps[:, h * 64:(h + 1) * 64],
                                    1.0,
                                    bv_bc[:, h * 64:(h + 1) * 64],
                                    op0=mult,
                                    op1=add,
                                )

                    # k -> q0 -> v(ns0, ns1); v(ns2, ns3) overlap into the
                    # first main block as filler
                    for ns in range(4):
                        proj_qk(kT, wk_sb, bk_sb, khT, ns)
                    for kc in range(8):
                        nc.scalar.dma_start(
                            wq_sb[:, kc * 256:(kc + 1) * 256],
                            wq[kc * 128:(kc + 1) * 128, :],
                        )
                    for kc in range(8):
                        nc.scalar.dma_start(
                            wv_sb[:, kc * 256:(kc + 1) * 256],
                            wv[kc * 128:(kc + 1) * 128, :],
                        )
                    proj_qk(qT, wq_sb, bq_sb, qhT, 0)
                    for ns in range(2):
                        for _ in vproj_chunk(ns, PV):
                            pass
                    for p in range(2):
                        nc.scalar.dma_start(wo_sb[p][:], wo[p * 128:(p + 1) * 128, :])

                # ---- Main loop over (qs, p) blocks ----
                with tc.tile_pool(name="psS", bufs=1, space="PSUM") as PS, \
                     tc.tile_pool(name="psC", bufs=1, space="PSUM") as PC, \
                     tc.tile_pool(name="psDF", bufs=2, space="PSUM") as PD, \
                     tc.tile_pool(name="expA", bufs=3) as EA, \
                     tc.tile_pool(name="expD", bufs=3) as ED, \
                     tc.tile_pool(name="nrm", bufs=4) as NP, \
                     tc.tile_pool(name="outP", bufs=2) as OP:

                    def qh_filler(ns):
                        # DVE is exp-busy mid-loop: qh loads on sync only
                        xt = load_x(qT, ns, engs=(nc.sync,))
                        for p in range(2):
                            ps = PD.tile((128, 512), fp32, name="dps")
                            for kc in range(8):
                                nc.tensor.matmul(
                                    ps[:],
                                    wq_sb[:, kc * 256 + p * 128:kc * 256 + (p + 1) * 128],
                                    xt[:, kc * 512:(kc + 1) * 512],
                                    start=(kc == 0),
                                    stop=(kc == 7),
                                )
                                yield
                            nc.vector.tensor_scalar_add(
                                qhT[p][:, ns * 512:(ns + 1) * 512], ps[:], bq_sb[:, p:p + 1]
                            )

                    def d_filler(qs, tail=False):
                        for qq in range(4):
                            qc0 = qs * 512 + qq * 128
                            osb = OP.tile((128, D), fp32, name="osb")
                            for nsd in range(2):
                                dps = PD.tile((128, 512), fp32, name="dps")
                                for p in range(2):
                                    nc.tensor.matmul(
                                        dps[:],
                                        outnT[p][:, qc0:qc0 + 128],
                                        wo_sb[p][:, nsd * 512:(nsd + 1) * 512],
                                        start=(p == 0),
                                        stop=(p == 1),
                                    )
                                    yield
                                # psum->sbuf copy: ACT+DVE alternating at the
                                # tail (both idle), DVE mid-loop (ACT is
                                # exp-heavy)
                                if tail and nsd == 0:
                                    nc.scalar.copy(
                                        osb[:, nsd * 512:(nsd + 1) * 512], dps[:]
                                    )
                                else:
                                    nc.vector.tensor_scalar_add(
                                        osb[:, nsd * 512:(nsd + 1) * 512], dps[:], 0.0
                                    )
                            # row split: each dma_start lands on one DMA
                            # engine (~22.5 GB/s), so a single 512KB
                            # transfer would drain for ~23us
                            for rr in range(4):
                                eng = nc.scalar if (tail and rr % 2) else nc.sync
                                eng.dma_start(
                                    out[qc0 + rr * 32:qc0 + (rr + 1) * 32, :],
                                    osb[rr * 32:(rr + 1) * 32, :],
                                )

                    def chain(*gens):
                        for g in gens:
                            yield from g

                    def emit_scores(p, qs, jp, sps4):
                        # 4 MMs interleaved (e jc0), (o jc0), (e jc1), (o jc1):
                        # even head rows 0-63, odd rows 64-127 -> pairwise
                        # concurrent row-tiles. Every quarter is its own
                        # 1-bank psum tile freed by its own exp call, so all
                        # four MMs are ready at slot start (full pairing).
                        for half in range(2):
                            jc = jp * 2 + half
                            for po in range(2):  # 0 = even head, 1 = odd head
                                off = po * 64
                                nc.tensor.matmul(
                                    sps4[po * 2 + half][:],
                                    khT[p][off:off + 64, jc * 128:(jc + 1) * 128],
                                    qhT[p][off:off + 64, qs * 512:(qs + 1) * 512],
                                    start=True,
                                    stop=True,
                                )

                    def emit_exp(sps4, ex4):
                        # ACT: head-even halves + head-odd half1 (exact);
                        # DVE: head-odd half0 via Schraudolph bits
                        # uint16(x*A+B) viewed as bf16 (~1.8% rel std).
                        nc.scalar.activation(
                            ex4[0][:], sps4[0][:], Exp, bias=0.0, scale=0.125)
                        nc.vector.tensor_scalar(
                            ex4[2][:].bitcast(u16), sps4[2][:],
                            A16 * 0.125, B16, op0=mult, op1=add)
                        nc.scalar.activation(
                            ex4[1][:], sps4[1][:], Exp, bias=0.0, scale=0.125)
                        nc.scalar.activation(
                            ex4[3][:], sps4[3][:], Exp, bias=0.0, scale=0.125)

                    def emit_c(p, jp, cpse, cpso, ex4):
                        for half in range(2):
                            jc = jp * 2 + half
                            nc.tensor.matmul(
                                cpse[:],
                                vh[2 * p][:, jc * 65:(jc + 1) * 65],
                                ex4[half][:],
                                start=(jc == 0),
                                stop=(jc == 15),
                            )
                            nc.tensor.matmul(
                                cpso[:],
                                vh[2 * p + 1][:, jc * 65:(jc + 1) * 65],
                                ex4[2 + half][:],
                                start=(jc == 0),
                                stop=(jc == 15),
                            )

                    norm_q = []

                    def normalize(p, po, qs, cps):
                        off = po * 64
                        den = NP.tile((1, 512), fp32, name="den")
                        # recip needs SBUF input; denom row copy on ACT
                        nc.scalar.copy(den[:], cps[64:65, :])
                        rec = NP.tile((1, 512), fp32, name="rec")
                        nc.vector.reciprocal_approx_fast(rec[:], den[:])
                        rbc = NP.tile((64, 512), fp32, name="rbc")
                        nc.gpsimd.partition_broadcast(rbc[:], rec[:])

                        # the final multiplies are deferred one consume so
                        # they don't bunch up in the DVE queue ahead of the
                        # next slot's exp (they still precede the next
                        # block's C writes into the same cps banks)
                        def mult_fin():
                            nc.vector.scalar_tensor_tensor(
                                outnT[p][off:off + 64, qs * 512:(qs + 1) * 512],
                                cps[0:64, :],
                                1.0,
                                rbc[:],
                                op0=mult,
                                op1=mult,
                            )
                        norm_q.append(mult_fin)

                    # block schedule: (qs, p) with fillers
                    blocks = [(qs, p) for qs in range(4) for p in range(2)]
                    fillers = {
                        0: chain(vproj_chunk(2, PD), vproj_chunk(3, PD)),
                        1: chain(qh_filler(1)),
                        2: chain(qh_filler(2)),
                        3: chain(d_filler(0)),
                        4: chain(qh_filler(3)),
                        5: chain(d_filler(1)),
                        7: chain(d_filler(2)),
                    }

                    pending = [None]

                    def consume_pending():
                        # deferred normalize multiplies first: they must
                        # precede any new C matmul into the recycled banks
                        while norm_q:
                            norm_q.pop(0)()
                        if pending[0] is not None:
                            (pp, pqs, pjp, pcpse, pcpso, pexs) = pending[0]
                            emit_c(pp, pjp, pcpse, pcpso, pexs)
                            if pjp == 7:
                                normalize(pp, 0, pqs, pcpse)
                                normalize(pp, 1, pqs, pcpso)
                            pending[0] = None

                    for bi, (qs, p) in enumerate(blocks):
                        filler = fillers.get(bi)
                        # consume before re-allocating from the 2-deep cps
                        # pool: the pending block's C/normalize must be
                        # emitted before its banks are handed out again
                        consume_pending()
                        cpse = PC.tile((65, 512), fp32, name="cpse")
                        cpso = PC.tile((65, 512), fp32, name="cpso")
                        for jp in range(8):
                            sps4 = [PS.tile((128, 512), fp32, name=f"sps{i}")
                                    for i in range(4)]
                            emit_scores(p, qs, jp, sps4)
                            ex4 = [
                                EA.tile((128, 512), bf16, name="exe0"),
                                EA.tile((128, 512), bf16, name="exe1"),
                                ED.tile((128, 512), bf16, name="exo0"),
                                EA.tile((128, 512), bf16, name="exo1"),
                            ]
                            emit_exp(sps4, ex4)
                            consume_pending()
                            if filler is not None:
                                next(filler, None)
                                next(filler, None)
                            pending[0] = (p, qs, jp, cpse, cpso, ex4)
                        if filler is not None:
                            for _ in filler:
                                pass
                    consume_pending()
                    while norm_q:
                        norm_q.pop(0)()
                    for _ in d_filler(3, tail=True):
                        pass

    nc.compile()
    return nc


def _get_nc():
    global _NC
    if _NC is None:
        _NC = _build()
    return _NC


def run(inputs, trace=False, trace_cores=None):
    from concourse.bass_utils import run_bass_kernel_spmd

    q = np.asarray(inputs["q"], np.float32)
    k = np.asarray(inputs["k"], np.float32)
    v = np.asarray(inputs["v"], np.float32)
    w_q = np.asarray(inputs["w_q"], np.float32)
    w_k = np.asarray(inputs["w_k"], np.float32)
    w_v = np.asarray(inputs["w_v"], np.float32)
    w_out = np.asarray(inputs["w_out"], np.float32)
    b_q = np.asarray(inputs["b_q"], np.float32)
    b_k = np.asarray(inputs["b_k"], np.float32)
    b_v = np.asarray(inputs["b_v"], np.float32)
    b_out = np.asarray(inputs["b_out"], np.float32)

    import ml_dtypes
    bf16 = ml_dtypes.bfloat16

    xT = {b: {} for b in range(B)}
    for b in range(B):
        xT[b]["qT"] = np.ascontiguousarray(q[b].T.astype(bf16))
        xT[b]["kT"] = np.ascontiguousarray(k[b].T.astype(bf16))
        xT[b]["vT"] = np.ascontiguousarray(v[b].T.astype(bf16))

    in_maps = []
    for c in range(N_CORES):
        b, hq = c // 4, c % 4
        rows = slice(hq * 256, (hq + 1) * 256)
        in_maps.append({
            "qT": xT[b]["qT"],
            "kT": xT[b]["kT"],
            "vT": xT[b]["vT"],
            "wq": np.ascontiguousarray(w_q[rows, :].T.astype(bf16)),
            "wk": np.ascontiguousarray(w_k[rows, :].T.astype(bf16)),
            "wv": np.ascontiguousarray(w_v[rows, :].T.astype(bf16)),
            "wo": np.ascontiguousarray(w_out[:, rows].T.astype(bf16)),
            "bq": np.ascontiguousarray(b_q[rows].reshape(256, 1)),
            "bk": np.ascontiguousarray(b_k[rows].reshape(256, 1)),
            "bv": np.ascontiguousarray(b_v[rows].reshape(1, 256)),
        })

    nc = _get_nc()
    res = run_bass_kernel_spmd(
        nc, in_maps, core_ids=list(range(N_CORES)), trace=trace,
        trace_cores=trace_cores,
    )
    full = np.zeros((B, S, D), np.float32)
    for c in range(N_CORES):
        full[c // 4] += np.asarray(res.results[c]["out"])
    full += b_out.reshape(1, 1, D)
    return full, res.exec_time_ns


def kernel(**inputs):
    return run(inputs, trace=False)[0]
